# revision 3
# baseline (speedup 1.0000x reference)
"""Trainium2 Bass kernel for nn_DoubleNet (topk_masking).

Computation (see reference):
  5 hidden layers: h = relu(h @ (m1_l - m2_l).T + 2*b_l)   [8192, 4096]
  output layer:    h = relu(h @ (m1_o - m2_o).T + 2*b_o)   [8192, 2]
  final:           y = h @ w_last.T                        [8192, 1]
where m1/m2 are top-50% masks of |scores| (exact argsort tie semantics).

Strategy:
  - Masks are x-independent: computed exactly on host with an O(n)
    partition + stable tie-fix that matches jnp.argsort(stable) bit-exactly.
  - Data-parallel: batch 8192 split as 1024 rows per NeuronCore (8 cores).
  - Weights w = m1 - m2 in {-1, 0, +1} are exact in bf16/fp16. In the
    default "split2" mode activations are carried as a hi/lo bf16 pair
    (h = hi + lo), giving ~fp32 accuracy with two bf16 matmul passes
    (products are exact; PSUM accumulates fp32).
  - Per core, activations stay resident in SBUF (chunk of 512 batch cols
    at a time through all layers); weights stream from HBM, double-buffered.

MODE:
  "split2" (default): hi/lo bf16, 2 matmul passes, ~2.5e-5 rel err.
  "fp16": single fp16 pass with per-layer 2^-6 rescale (activations grow
          ~45x/layer, fp16 max is 65504), ~2.2e-3 rel err, half the
          matmul work.
"""

import sys

for _p in ("/opt/trn_rl_repo", "/root/.axon_site/_ro/trn_rl_repo"):
    if _p not in sys.path:
        sys.path.insert(0, _p)

import numpy as np
import ml_dtypes

import concourse.mybir as mybir
import concourse.tile as tile
from concourse import bacc
from concourse.bass_utils import run_bass_kernel_spmd

BF = ml_dtypes.bfloat16
F16 = np.float16
BF16 = mybir.dt.bfloat16
FP16 = mybir.dt.float16
F32 = mybir.dt.float32

P = 128          # partitions
N_CORES = 8
B = 8192         # total batch
D = 4096         # width
L = 5            # hidden layers
KEEP = 0.5
NT = D // P      # 32 d/ko tiles
BC = B // N_CORES  # 1024 batch rows per core
NB = 512         # matmul free dim (one PSUM bank of fp32)
CH = BC // NB    # 2 chunks per core

MODE = "fp16"  # "split2" | "fp16" | "hybrid"
FP16_SHIFT = 6   # fp16/hybrid: h_l is carried scaled by 2^(-FP16_SHIFT*l)
LOSC = 8192.0    # hybrid: lo residual is stored as fp8 scaled by 2^13

_BUILD_CACHE = {}


def _subnet_mask(scores: np.ndarray) -> np.ndarray:
    """Exact replica of reference.get_subnet(|scores|) forward value.

    Zero the j smallest |scores| (ties at the threshold broken by flat
    index order, matching stable argsort), one elsewhere.
    """
    flat = np.abs(scores.astype(np.float32, copy=False)).ravel()
    n = flat.size
    j = int((1.0 - KEEP) * n)
    if j == 0:
        return np.ones(scores.shape, np.float32)
    thr = np.partition(flat, j - 1)[j - 1]
    mask = (flat > thr).astype(np.float32)
    c_lt = int((flat < thr).sum())
    idx_eq = np.flatnonzero(flat == thr)
    n_zero_eq = j - c_lt
    assert 0 <= n_zero_eq <= idx_eq.size
    mask[idx_eq[n_zero_eq:]] = 1.0
    return mask.reshape(scores.shape)


def _build_split2():
    nc = bacc.Bacc("TRN2", target_bir_lowering=False, debug=False)
    xt_d = nc.dram_tensor("xt", [CH, 2, P, NT * NB], BF16, kind="ExternalInput").ap()
    wt_d = nc.dram_tensor("wt", [L, NT, P, NT * P], BF16, kind="ExternalInput").ap()
    bias_d = nc.dram_tensor("biasd", [P, L * NT], F32, kind="ExternalInput").ap()
    wo_d = nc.dram_tensor("wo", [P, NT * 2], BF16, kind="ExternalInput").ap()
    z_d = nc.dram_tensor("z", [2, BC], F32, kind="ExternalOutput").ap()

    with tile.TileContext(nc) as tc:
        with (
            tc.tile_pool(name="acts", bufs=1) as acts,
            tc.tile_pool(name="wpool", bufs=3) as wpool,
            tc.tile_pool(name="tmp", bufs=3) as tpool,
            tc.tile_pool(name="const", bufs=1) as cpool,
            tc.tile_pool(name="psum", bufs=4, space="PSUM") as ppool,
            tc.tile_pool(name="zpsum", bufs=2, space="PSUM") as zppool,
            tc.tile_pool(name="zsb", bufs=2) as zsbpool,
        ):
            A_hi = acts.tile([P, NT * NB], BF16, name="A_hi")
            A_lo = acts.tile([P, NT * NB], BF16, name="A_lo")
            B_hi = acts.tile([P, NT * NB], BF16, name="B_hi")
            B_lo = acts.tile([P, NT * NB], BF16, name="B_lo")
            bias_sb = cpool.tile([P, L * NT], F32, name="bias_sb")
            wo_sb = cpool.tile([P, NT * 2], BF16, name="wo_sb")
            nc.sync.dma_start(bias_sb[:], bias_d[:])
            nc.sync.dma_start(wo_sb[:], wo_d[:])

            for c in range(CH):
                nc.sync.dma_start(A_hi[:], xt_d[c, 0])
                nc.sync.dma_start(A_lo[:], xt_d[c, 1])
                for l in range(L):
                    ain_hi, ain_lo = (A_hi, A_lo) if l % 2 == 0 else (B_hi, B_lo)
                    aout_hi, aout_lo = (B_hi, B_lo) if l % 2 == 0 else (A_hi, A_lo)
                    for ko in range(NT):
                        slab = wpool.tile([P, NT * P], BF16, name="wslab")
                        nc.sync.dma_start(slab[:], wt_d[l, ko])
                        pt = ppool.tile([P, NB], F32, name="pt")
                        for d in range(NT):
                            lhsT = slab[:, d * P : (d + 1) * P]
                            nc.tensor.matmul(
                                pt[:], lhsT, ain_hi[:, d * NB : (d + 1) * NB],
                                start=(d == 0), stop=False,
                            )
                            nc.tensor.matmul(
                                pt[:], lhsT, ain_lo[:, d * NB : (d + 1) * NB],
                                start=False, stop=(d == NT - 1),
                            )
                        tmp = tpool.tile([P, NB], F32, name="tmp")
                        nc.scalar.activation(
                            tmp[:], pt[:], mybir.ActivationFunctionType.Relu,
                            bias=bias_sb[:, l * NT + ko : l * NT + ko + 1], scale=1.0,
                        )
                        nc.vector.tensor_copy(
                            aout_hi[:, ko * NB : (ko + 1) * NB], tmp[:]
                        )
                        nc.vector.tensor_sub(
                            aout_lo[:, ko * NB : (ko + 1) * NB],
                            tmp[:],
                            aout_hi[:, ko * NB : (ko + 1) * NB],
                        )

                # output layer: z[2, NB] = w_o @ h5 (pre-bias, pre-relu)
                hout_hi, hout_lo = (A_hi, A_lo) if L % 2 == 0 else (B_hi, B_lo)
                zp = zppool.tile([2, NB], F32, name="zp")
                for t in range(NT):
                    lhsT = wo_sb[:, t * 2 : (t + 1) * 2]
                    nc.tensor.matmul(
                        zp[:], lhsT, hout_hi[:, t * NB : (t + 1) * NB],
                        start=(t == 0), stop=False,
                    )
                    nc.tensor.matmul(
                        zp[:], lhsT, hout_lo[:, t * NB : (t + 1) * NB],
                        start=False, stop=(t == NT - 1),
                    )
                zs = zsbpool.tile([2, NB], F32, name="zs")
                nc.vector.tensor_copy(zs[:], zp[:])
                nc.sync.dma_start(z_d[:, c * NB : (c + 1) * NB], zs[:])

    nc.compile()
    return nc


def _build_fp16():
    nc = bacc.Bacc("TRN2", target_bir_lowering=False, debug=False)
    xt_d = nc.dram_tensor("xt", [CH, P, NT * NB], FP16, kind="ExternalInput").ap()
    wt_d = nc.dram_tensor("wt", [L, NT, P, NT * P], FP16, kind="ExternalInput").ap()
    bias_d = nc.dram_tensor("biasd", [P, L * NT], F32, kind="ExternalInput").ap()
    wo_d = nc.dram_tensor("wo", [P, NT * 2], FP16, kind="ExternalInput").ap()
    z_d = nc.dram_tensor("z", [2, BC], F32, kind="ExternalOutput").ap()
    sc = float(2.0 ** (-FP16_SHIFT))

    with tile.TileContext(nc) as tc:
        with (
            tc.tile_pool(name="acts", bufs=1) as acts,
            tc.tile_pool(name="wpool", bufs=3) as wpool,
            tc.tile_pool(name="const", bufs=1) as cpool,
            tc.tile_pool(name="psum", bufs=4, space="PSUM") as ppool,
            tc.tile_pool(name="zpsum", bufs=2, space="PSUM") as zppool,
            tc.tile_pool(name="zsb", bufs=2) as zsbpool,
        ):
            # Both chunks' inputs prefetched up-front into dedicated buffers
            # (X0/X1) so the chunk-1 load is fully hidden under chunk-0 compute.
            X = [acts.tile([P, NT * NB], FP16, name=f"X{c}") for c in range(CH)]
            A = acts.tile([P, NT * NB], FP16, name="A")
            Bt = acts.tile([P, NT * NB], FP16, name="Bt")
            bias_sb = cpool.tile([P, L * NT], F32, name="bias_sb")
            wo_sb = cpool.tile([P, NT * 2], FP16, name="wo_sb")
            nc.sync.dma_start(bias_sb[:], bias_d[:])
            nc.sync.dma_start(wo_sb[:], wo_d[:])
            for c in range(CH):
                nc.sync.dma_start(X[c][:], xt_d[c])

            for c in range(CH):
                for l in range(L):
                    ain = X[c] if l == 0 else (Bt if l % 2 == 1 else A)
                    aout = Bt if l % 2 == 0 else A
                    for ko in range(NT):
                        slab = wpool.tile([P, NT * P], FP16, name="wslab")
                        nc.sync.dma_start(slab[:], wt_d[l, ko])
                        pt = ppool.tile([P, NB], F32, name="pt")
                        for d in range(NT):
                            nc.tensor.matmul(
                                pt[:], slab[:, d * P : (d + 1) * P],
                                ain[:, d * NB : (d + 1) * NB],
                                start=(d == 0), stop=(d == NT - 1),
                            )
                        # g_{l+1} = relu(2^-S * psum + 2*b*2^(-S(l+1))), fp16 out
                        nc.scalar.activation(
                            aout[:, ko * NB : (ko + 1) * NB], pt[:],
                            mybir.ActivationFunctionType.Relu,
                            bias=bias_sb[:, l * NT + ko : l * NT + ko + 1], scale=sc,
                        )

                hout = A if L % 2 == 0 else Bt
                zp = zppool.tile([2, NB], F32, name="zp")
                for t in range(NT):
                    nc.tensor.matmul(
                        zp[:], wo_sb[:, t * 2 : (t + 1) * 2],
                        hout[:, t * NB : (t + 1) * NB],
                        start=(t == 0), stop=(t == NT - 1),
                    )
                zs = zsbpool.tile([2, NB], F32, name="zs")
                nc.vector.tensor_copy(zs[:], zp[:])
                nc.sync.dma_start(z_d[:, c * NB : (c + 1) * NB], zs[:])

    nc.compile()
    return nc


def _build_hybrid():
    """fp16 hi + fp8e4m3 lo (DoubleRow) with 2^-6/layer activation rescale.

    h = hi + lo/LOSC; hi pass: 32 fp16 matmuls; lo pass: 16 fp8 DoubleRow
    matmuls (2 k-tiles each) into a separate PSUM bank, combined at evict.
    """
    FP8 = mybir.dt.float8e4
    nc = bacc.Bacc("TRN2", target_bir_lowering=False, debug=False)
    xth_d = nc.dram_tensor("xth", [CH, P, NT * NB], FP16, kind="ExternalInput").ap()
    xtl_d = nc.dram_tensor("xtl", [CH, P, NT * NB], FP8, kind="ExternalInput").ap()
    wt16_d = nc.dram_tensor("wt16", [L, NT, P, NT * P], FP16, kind="ExternalInput").ap()
    wt8_d = nc.dram_tensor("wt8", [L, NT, P, NT * P], FP8, kind="ExternalInput").ap()
    bias_d = nc.dram_tensor("biasd", [P, L * NT], F32, kind="ExternalInput").ap()
    wo16_d = nc.dram_tensor("wo16", [P, NT * 2], FP16, kind="ExternalInput").ap()
    wo8_d = nc.dram_tensor("wo8", [P, NT * 2], FP8, kind="ExternalInput").ap()
    z_d = nc.dram_tensor("z", [2, BC], F32, kind="ExternalOutput").ap()
    sc = float(2.0 ** (-FP16_SHIFT))

    with tile.TileContext(nc) as tc:
        with (
            tc.tile_pool(name="acts", bufs=1) as acts,
            tc.tile_pool(name="w16pool", bufs=3) as w16pool,
            tc.tile_pool(name="w8pool", bufs=3) as w8pool,
            tc.tile_pool(name="tmp", bufs=3) as tpool,
            tc.tile_pool(name="const", bufs=1) as cpool,
            tc.tile_pool(name="psumh", bufs=3, space="PSUM") as pph,
            tc.tile_pool(name="psuml", bufs=3, space="PSUM") as ppl,
            tc.tile_pool(name="zpsum", bufs=1, space="PSUM") as zppool,
            tc.tile_pool(name="zsb", bufs=2) as zsbpool,
        ):
            A_hi = acts.tile([P, NT * NB], FP16, name="A_hi")
            A_lo = acts.tile([P, NT * NB], FP8, name="A_lo")
            B_hi = acts.tile([P, NT * NB], FP16, name="B_hi")
            B_lo = acts.tile([P, NT * NB], FP8, name="B_lo")
            bias_sb = cpool.tile([P, L * NT], F32, name="bias_sb")
            wo16_sb = cpool.tile([P, NT * 2], FP16, name="wo16_sb")
            wo8_sb = cpool.tile([P, NT * 2], FP8, name="wo8_sb")
            nc.sync.dma_start(bias_sb[:], bias_d[:])
            nc.sync.dma_start(wo16_sb[:], wo16_d[:])
            nc.sync.dma_start(wo8_sb[:], wo8_d[:])

            for c in range(CH):
                nc.sync.dma_start(A_hi[:], xth_d[c])
                nc.sync.dma_start(A_lo[:], xtl_d[c])
                for l in range(L):
                    ain_hi, ain_lo = (A_hi, A_lo) if l % 2 == 0 else (B_hi, B_lo)
                    aout_hi, aout_lo = (B_hi, B_lo) if l % 2 == 0 else (A_hi, A_lo)
                    for ko in range(NT):
                        slab16 = w16pool.tile([P, NT * P], FP16, name="w16slab")
                        nc.sync.dma_start(slab16[:], wt16_d[l, ko])
                        slab8 = w8pool.tile([P, NT * P], FP8, name="w8slab")
                        nc.sync.dma_start(slab8[:], wt8_d[l, ko])
                        # NOTE: batched ordering (all fp16, then all DR) measures
                        # faster than hi,hi,lo interleave (219.4 vs 224.2 ns/MM):
                        # alternating Normal/DoubleRow perf modes thrashes the
                        # PE weight path more than the DR LDWEIGHTS costs.
                        pt = pph.tile([P, NB], F32, name="pt")
                        for d in range(NT):
                            nc.tensor.matmul(
                                pt[:], slab16[:, d * P : (d + 1) * P],
                                ain_hi[:, d * NB : (d + 1) * NB],
                                start=(d == 0), stop=(d == NT - 1),
                            )
                        plo = ppl.tile([P, NB], F32, name="plo")
                        for m in range(NT // 2):
                            lhsT = slab8[:, 2 * m * P : (2 * m + 2) * P].rearrange(
                                "p (j c) -> p j c", j=2
                            )
                            rhs = ain_lo[
                                :, 2 * m * NB : (2 * m + 2) * NB
                            ].rearrange("p (j b) -> p j b", j=2)
                            nc.tensor.matmul(
                                plo[:], lhsT, rhs,
                                start=(m == 0), stop=(m == NT // 2 - 1),
                                perf_mode=mybir.MatmulPerfMode.DoubleRow,
                            )
                        # combine + relu + re-split (t4s is relu result x LOSC)
                        t1 = tpool.tile([P, NB], F32, name="t1")
                        nc.scalar.mul(t1[:], plo[:], 1.0 / LOSC)
                        t2 = tpool.tile([P, NB], F32, name="t2")
                        nc.vector.tensor_add(t2[:], t1[:], pt[:])
                        t4s = tpool.tile([P, NB], F32, name="t4s")
                        nc.scalar.activation(
                            t4s[:], t2[:], mybir.ActivationFunctionType.Relu,
                            bias=bias_sb[:, l * NT + ko : l * NT + ko + 1],
                            scale=sc * LOSC,
                        )
                        nc.vector.tensor_scalar_mul(
                            aout_hi[:, ko * NB : (ko + 1) * NB], t4s[:], 1.0 / LOSC
                        )
                        nc.vector.scalar_tensor_tensor(
                            aout_lo[:, ko * NB : (ko + 1) * NB],
                            aout_hi[:, ko * NB : (ko + 1) * NB], -LOSC, t4s[:],
                            op0=mybir.AluOpType.mult, op1=mybir.AluOpType.add,
                        )

                hout_hi, hout_lo = (A_hi, A_lo) if L % 2 == 0 else (B_hi, B_lo)
                zph = zppool.tile([2, NB], F32, name="zph")
                for t in range(NT):
                    nc.tensor.matmul(
                        zph[:], wo16_sb[:, t * 2 : (t + 1) * 2],
                        hout_hi[:, t * NB : (t + 1) * NB],
                        start=(t == 0), stop=(t == NT - 1),
                    )
                zpl = zppool.tile([2, NB], F32, name="zpl")
                for t in range(NT):
                    nc.tensor.matmul(
                        zpl[:], wo8_sb[:, t * 2 : (t + 1) * 2],
                        hout_lo[:, t * NB : (t + 1) * NB],
                        start=(t == 0), stop=(t == NT - 1),
                    )
                zs1 = zsbpool.tile([2, NB], F32, name="zs1")
                nc.scalar.mul(zs1[:], zpl[:], 1.0 / LOSC)
                zs = zsbpool.tile([2, NB], F32, name="zs")
                nc.vector.tensor_add(zs[:], zs1[:], zph[:])
                nc.sync.dma_start(z_d[:, c * NB : (c + 1) * NB], zs[:])

    nc.compile()
    return nc


def _build_program(mode):
    if mode not in _BUILD_CACHE:
        _BUILD_CACHE[mode] = {
            "split2": _build_split2, "fp16": _build_fp16, "hybrid": _build_hybrid,
        }[mode]()
    return _BUILD_CACHE[mode]


def _split_hilo(a32: np.ndarray):
    hi = a32.astype(BF)
    lo = (a32 - hi.astype(np.float32)).astype(BF)
    return hi, lo


def _weights_layout(w: np.ndarray) -> np.ndarray:
    """[dout, din] f32 -> [NT(ko), P(p), NT*P(t*128+c)]."""
    return w.reshape(NT, P, NT, P).transpose(0, 3, 2, 1).reshape(NT, P, NT * P)


def _prepare_inputs(mode, x, scores1_h, scores2_h, bias_h, scores1_o, scores2_o):
    """Host-side: masks, weight/bias/x layouts for the device program.

    Returns a list of per-core in_maps (without only the core-varying xt)."""
    F8 = ml_dtypes.float8_e4m3
    wdt = BF if mode == "split2" else F16
    wt = np.empty((L, NT, P, NT * P), wdt)
    wt8 = np.empty((L, NT, P, NT * P), F8) if mode == "hybrid" else None
    for l in range(L):
        w = _subnet_mask(scores1_h[l]) - _subnet_mask(scores2_h[l])  # [dout, din]
        wl = _weights_layout(w)
        wt[l] = wl.astype(wdt)
        if wt8 is not None:
            wt8[l] = wl.astype(F8)

    b2 = 2.0 * bias_h.astype(np.float32)  # [L, D]
    if mode in ("fp16", "hybrid"):
        scales = (2.0 ** (-FP16_SHIFT * np.arange(1, L + 1, dtype=np.float32)))
        b2 = b2 * scales[:, None]
        if mode == "hybrid":
            b2 = b2 * LOSC
    bias_sb = np.ascontiguousarray(
        b2.reshape(L, NT, P).transpose(2, 0, 1).reshape(P, L * NT)
    )

    wo = _subnet_mask(scores1_o) - _subnet_mask(scores2_o)  # [2, D]
    wo_l = np.ascontiguousarray(
        wo.reshape(2, NT, P).transpose(2, 1, 0).reshape(P, NT * 2)
    )
    wo_sb = wo_l.astype(wdt)

    common = {}
    if mode == "hybrid":
        common = {"wt16": wt, "wt8": wt8, "biasd": bias_sb,
                  "wo16": wo_sb, "wo8": wo_l.astype(F8)}
    else:
        common = {"wt": wt, "biasd": bias_sb, "wo": wo_sb}

    in_maps = []
    for i in range(N_CORES):
        xT = x[i * BC : (i + 1) * BC].T.astype(np.float32)  # [D, BC]
        xr = xT.reshape(NT, P, CH, NB).transpose(2, 1, 0, 3).reshape(CH, P, NT * NB)
        xr = np.ascontiguousarray(xr)
        if mode == "split2":
            hi, lo = _split_hilo(xr)
            xt = {"xt": np.ascontiguousarray(np.stack([hi, lo], axis=1))}
        elif mode == "fp16":
            xt = {"xt": xr.astype(F16)}
        else:
            hi = xr.astype(F16)
            lo8 = ((xr - hi.astype(np.float32)) * LOSC).astype(F8)
            xt = {"xth": hi, "xtl": lo8}
        in_maps.append({**common, **xt})
    return in_maps


def kernel(x, scores1_h, scores2_h, bias_h, scores1_o, scores2_o, bias_o, w_last,
           _trace=False, _run_kwargs=None):
    x = np.asarray(x, np.float32)
    scores1_h = np.asarray(scores1_h, np.float32)
    scores2_h = np.asarray(scores2_h, np.float32)
    bias_h = np.asarray(bias_h, np.float32)
    scores1_o = np.asarray(scores1_o, np.float32)
    scores2_o = np.asarray(scores2_o, np.float32)
    bias_o = np.asarray(bias_o, np.float32)
    w_last = np.asarray(w_last, np.float32)

    in_maps = _prepare_inputs(
        MODE, x, scores1_h, scores2_h, bias_h, scores1_o, scores2_o
    )
    nc = _build_program(MODE)
    # Retry guard: very rare transient HW faults have been observed to produce
    # NaN output (z is tiny, so the check is free). Clean runs are bit-identical.
    for attempt in range(3):
        res = run_bass_kernel_spmd(
            nc, in_maps, core_ids=list(range(N_CORES)), trace=_trace,
            **(_run_kwargs or {}),
        )
        zs_all = np.stack([res.results[i]["z"] for i in range(N_CORES)])
        zbound = 1e11 if MODE == "split2" else 1e4  # z is 2^-30-scaled otherwise
        if np.isfinite(zs_all).all() and np.abs(zs_all).max() < zbound:
            break
        print(f"kernel: bad z detected (attempt {attempt}), retrying", file=sys.stderr)

    # host-side tail: relu(z.T + 2*b_o) @ w_last.T  (tiny: [8192, 2] -> [8192, 1])
    zscale = 1.0 if MODE == "split2" else float(2.0 ** (FP16_SHIFT * L))
    # (fp16/hybrid carry h5 scaled by 2^-30; z output is pre-bias, pre-relu)
    y = np.empty((B, 1), np.float32)
    for i in range(N_CORES):
        z = res.results[i]["z"].astype(np.float32)  # [2, BC]
        h = np.maximum(z.T * zscale + 2.0 * bias_o[None, :], 0.0).astype(np.float32)
        y[i * BC : (i + 1) * BC] = h @ w_last.T
    if _trace:
        kernel.last_results = res
    return y



# revision 4
# speedup vs baseline: 1.5628x; 1.5628x over previous
"""Trainium2 Bass kernel for nn_DoubleNet (topk_masking).

Computation (see reference):
  5 hidden layers: h = relu(h @ (m1_l - m2_l).T + 2*b_l)   [8192, 4096]
  output layer:    h = relu(h @ (m1_o - m2_o).T + 2*b_o)   [8192, 2]
  final:           y = h @ w_last.T                        [8192, 1]
where m1/m2 are top-50% masks of |scores| (exact argsort tie semantics).

Strategy:
  - Masks are x-independent: computed exactly on host with an O(n)
    partition + stable tie-fix that matches jnp.argsort(stable) bit-exactly.
  - Data-parallel: batch 8192 split as 1024 rows per NeuronCore (8 cores).
  - Weights w = m1 - m2 in {-1, 0, +1} are exact in bf16/fp16. In the
    default "split2" mode activations are carried as a hi/lo bf16 pair
    (h = hi + lo), giving ~fp32 accuracy with two bf16 matmul passes
    (products are exact; PSUM accumulates fp32).
  - Per core, activations stay resident in SBUF (chunk of 512 batch cols
    at a time through all layers); weights stream from HBM, double-buffered.

MODE:
  "split2" (default): hi/lo bf16, 2 matmul passes, ~2.5e-5 rel err.
  "fp16": single fp16 pass with per-layer 2^-6 rescale (activations grow
          ~45x/layer, fp16 max is 65504), ~2.2e-3 rel err, half the
          matmul work.
"""

import sys

for _p in ("/opt/trn_rl_repo", "/root/.axon_site/_ro/trn_rl_repo"):
    if _p not in sys.path:
        sys.path.insert(0, _p)

import numpy as np
import ml_dtypes

import concourse.mybir as mybir
import concourse.tile as tile
from concourse import bacc
from concourse.bass_utils import run_bass_kernel_spmd

BF = ml_dtypes.bfloat16
F16 = np.float16
BF16 = mybir.dt.bfloat16
FP16 = mybir.dt.float16
F32 = mybir.dt.float32

P = 128          # partitions
N_CORES = 8
B = 8192         # total batch
D = 4096         # width
L = 5            # hidden layers
KEEP = 0.5
NT = D // P      # 32 d/ko tiles
BC = B // N_CORES  # 1024 batch rows per core
NB = 512         # matmul free dim (one PSUM bank of fp32)
CH = BC // NB    # 2 chunks per core

MODE = "fp16"  # "split2" | "fp16" | "hybrid"
FP16_SHIFT = 6   # fp16/hybrid: h_l is carried scaled by 2^(-FP16_SHIFT*l)
LOSC = 8192.0    # hybrid: lo residual is stored as fp8 scaled by 2^13

_BUILD_CACHE = {}


def _subnet_mask(scores: np.ndarray) -> np.ndarray:
    """Exact replica of reference.get_subnet(|scores|) forward value.

    Zero the j smallest |scores| (ties at the threshold broken by flat
    index order, matching stable argsort), one elsewhere.
    """
    flat = np.abs(scores.astype(np.float32, copy=False)).ravel()
    n = flat.size
    j = int((1.0 - KEEP) * n)
    if j == 0:
        return np.ones(scores.shape, np.float32)
    thr = np.partition(flat, j - 1)[j - 1]
    mask = (flat > thr).astype(np.float32)
    c_lt = int((flat < thr).sum())
    idx_eq = np.flatnonzero(flat == thr)
    n_zero_eq = j - c_lt
    assert 0 <= n_zero_eq <= idx_eq.size
    mask[idx_eq[n_zero_eq:]] = 1.0
    return mask.reshape(scores.shape)


def _build_split2():
    nc = bacc.Bacc("TRN2", target_bir_lowering=False, debug=False)
    xt_d = nc.dram_tensor("xt", [CH, 2, P, NT * NB], BF16, kind="ExternalInput").ap()
    wt_d = nc.dram_tensor("wt", [L, NT, P, NT * P], BF16, kind="ExternalInput").ap()
    bias_d = nc.dram_tensor("biasd", [P, L * NT], F32, kind="ExternalInput").ap()
    wo_d = nc.dram_tensor("wo", [P, NT * 2], BF16, kind="ExternalInput").ap()
    z_d = nc.dram_tensor("z", [2, BC], F32, kind="ExternalOutput").ap()

    with tile.TileContext(nc) as tc:
        with (
            tc.tile_pool(name="acts", bufs=1) as acts,
            tc.tile_pool(name="wpool", bufs=3) as wpool,
            tc.tile_pool(name="tmp", bufs=3) as tpool,
            tc.tile_pool(name="const", bufs=1) as cpool,
            tc.tile_pool(name="psum", bufs=4, space="PSUM") as ppool,
            tc.tile_pool(name="zpsum", bufs=2, space="PSUM") as zppool,
            tc.tile_pool(name="zsb", bufs=2) as zsbpool,
        ):
            A_hi = acts.tile([P, NT * NB], BF16, name="A_hi")
            A_lo = acts.tile([P, NT * NB], BF16, name="A_lo")
            B_hi = acts.tile([P, NT * NB], BF16, name="B_hi")
            B_lo = acts.tile([P, NT * NB], BF16, name="B_lo")
            bias_sb = cpool.tile([P, L * NT], F32, name="bias_sb")
            wo_sb = cpool.tile([P, NT * 2], BF16, name="wo_sb")
            nc.sync.dma_start(bias_sb[:], bias_d[:])
            nc.sync.dma_start(wo_sb[:], wo_d[:])

            for c in range(CH):
                nc.sync.dma_start(A_hi[:], xt_d[c, 0])
                nc.sync.dma_start(A_lo[:], xt_d[c, 1])
                for l in range(L):
                    ain_hi, ain_lo = (A_hi, A_lo) if l % 2 == 0 else (B_hi, B_lo)
                    aout_hi, aout_lo = (B_hi, B_lo) if l % 2 == 0 else (A_hi, A_lo)
                    for ko in range(NT):
                        slab = wpool.tile([P, NT * P], BF16, name="wslab")
                        nc.sync.dma_start(slab[:], wt_d[l, ko])
                        pt = ppool.tile([P, NB], F32, name="pt")
                        for d in range(NT):
                            lhsT = slab[:, d * P : (d + 1) * P]
                            nc.tensor.matmul(
                                pt[:], lhsT, ain_hi[:, d * NB : (d + 1) * NB],
                                start=(d == 0), stop=False,
                            )
                            nc.tensor.matmul(
                                pt[:], lhsT, ain_lo[:, d * NB : (d + 1) * NB],
                                start=False, stop=(d == NT - 1),
                            )
                        tmp = tpool.tile([P, NB], F32, name="tmp")
                        nc.scalar.activation(
                            tmp[:], pt[:], mybir.ActivationFunctionType.Relu,
                            bias=bias_sb[:, l * NT + ko : l * NT + ko + 1], scale=1.0,
                        )
                        nc.vector.tensor_copy(
                            aout_hi[:, ko * NB : (ko + 1) * NB], tmp[:]
                        )
                        nc.vector.tensor_sub(
                            aout_lo[:, ko * NB : (ko + 1) * NB],
                            tmp[:],
                            aout_hi[:, ko * NB : (ko + 1) * NB],
                        )

                # output layer: z[2, NB] = w_o @ h5 (pre-bias, pre-relu)
                hout_hi, hout_lo = (A_hi, A_lo) if L % 2 == 0 else (B_hi, B_lo)
                zp = zppool.tile([2, NB], F32, name="zp")
                for t in range(NT):
                    lhsT = wo_sb[:, t * 2 : (t + 1) * 2]
                    nc.tensor.matmul(
                        zp[:], lhsT, hout_hi[:, t * NB : (t + 1) * NB],
                        start=(t == 0), stop=False,
                    )
                    nc.tensor.matmul(
                        zp[:], lhsT, hout_lo[:, t * NB : (t + 1) * NB],
                        start=False, stop=(t == NT - 1),
                    )
                zs = zsbpool.tile([2, NB], F32, name="zs")
                nc.vector.tensor_copy(zs[:], zp[:])
                nc.sync.dma_start(z_d[:, c * NB : (c + 1) * NB], zs[:])

    nc.compile()
    return nc


def _build_fp16():
    nc = bacc.Bacc("TRN2", target_bir_lowering=False, debug=False)
    xt_d = nc.dram_tensor("xt", [CH, P, NT * NB], FP16, kind="ExternalInput").ap()
    wt_d = nc.dram_tensor("wt", [L, NT, P, NT * P], FP16, kind="ExternalInput").ap()
    bias_d = nc.dram_tensor("biasd", [P, L * NT], F32, kind="ExternalInput").ap()
    wo_d = nc.dram_tensor("wo", [P, NT * 2], FP16, kind="ExternalInput").ap()
    z_d = nc.dram_tensor("z", [2, BC], F32, kind="ExternalOutput").ap()
    sc = float(2.0 ** (-FP16_SHIFT))

    with tile.TileContext(nc) as tc:
        with (
            tc.tile_pool(name="acts", bufs=1) as acts,
            tc.tile_pool(name="wpool", bufs=3) as wpool,
            tc.tile_pool(name="const", bufs=1) as cpool,
            tc.tile_pool(name="psum", bufs=4, space="PSUM") as ppool,
            tc.tile_pool(name="zpsum", bufs=2, space="PSUM") as zppool,
            tc.tile_pool(name="zsb", bufs=2) as zsbpool,
        ):
            # Both chunks' inputs prefetched up-front into dedicated buffers
            # (X0/X1) so the chunk-1 load is fully hidden under chunk-0 compute.
            X = [acts.tile([P, NT * NB], FP16, name=f"X{c}") for c in range(CH)]
            A = acts.tile([P, NT * NB], FP16, name="A")
            Bt = acts.tile([P, NT * NB], FP16, name="Bt")
            bias_sb = cpool.tile([P, L * NT], F32, name="bias_sb")
            wo_sb = cpool.tile([P, NT * 2], FP16, name="wo_sb")
            nc.sync.dma_start(bias_sb[:], bias_d[:])
            nc.sync.dma_start(wo_sb[:], wo_d[:])
            for c in range(CH):
                nc.sync.dma_start(X[c][:], xt_d[c])

            for c in range(CH):
                for l in range(L):
                    ain = X[c] if l == 0 else (Bt if l % 2 == 1 else A)
                    aout = Bt if l % 2 == 0 else A
                    for ko in range(NT):
                        slab = wpool.tile([P, NT * P], FP16, name="wslab")
                        nc.sync.dma_start(slab[:], wt_d[l, ko])
                        pt = ppool.tile([P, NB], F32, name="pt")
                        for d in range(NT):
                            nc.tensor.matmul(
                                pt[:], slab[:, d * P : (d + 1) * P],
                                ain[:, d * NB : (d + 1) * NB],
                                start=(d == 0), stop=(d == NT - 1),
                            )
                        # g_{l+1} = relu(2^-S * psum + 2*b*2^(-S(l+1))), fp16 out
                        nc.scalar.activation(
                            aout[:, ko * NB : (ko + 1) * NB], pt[:],
                            mybir.ActivationFunctionType.Relu,
                            bias=bias_sb[:, l * NT + ko : l * NT + ko + 1], scale=sc,
                        )

                hout = A if L % 2 == 0 else Bt
                zp = zppool.tile([2, NB], F32, name="zp")
                for t in range(NT):
                    nc.tensor.matmul(
                        zp[:], wo_sb[:, t * 2 : (t + 1) * 2],
                        hout[:, t * NB : (t + 1) * NB],
                        start=(t == 0), stop=(t == NT - 1),
                    )
                zs = zsbpool.tile([2, NB], F32, name="zs")
                nc.vector.tensor_copy(zs[:], zp[:])
                nc.sync.dma_start(z_d[:, c * NB : (c + 1) * NB], zs[:])

    nc.compile()
    return nc


def _build_hybrid():
    """fp16 hi + fp8e4m3 lo (DoubleRow) with 2^-6/layer activation rescale.

    h = hi + lo/LOSC; hi pass: 32 fp16 matmuls; lo pass: 16 fp8 DoubleRow
    matmuls (2 k-tiles each) into a separate PSUM bank, combined at evict.
    """
    FP8 = mybir.dt.float8e4
    nc = bacc.Bacc("TRN2", target_bir_lowering=False, debug=False)
    xth_d = nc.dram_tensor("xth", [CH, P, NT * NB], FP16, kind="ExternalInput").ap()
    xtl_d = nc.dram_tensor("xtl", [CH, P, NT * NB], FP8, kind="ExternalInput").ap()
    wt16_d = nc.dram_tensor("wt16", [L, NT, P, NT * P], FP16, kind="ExternalInput").ap()
    wt8_d = nc.dram_tensor("wt8", [L, NT, P, NT * P], FP8, kind="ExternalInput").ap()
    bias_d = nc.dram_tensor("biasd", [P, L * NT], F32, kind="ExternalInput").ap()
    wo16_d = nc.dram_tensor("wo16", [P, NT * 2], FP16, kind="ExternalInput").ap()
    wo8_d = nc.dram_tensor("wo8", [P, NT * 2], FP8, kind="ExternalInput").ap()
    z_d = nc.dram_tensor("z", [2, BC], F32, kind="ExternalOutput").ap()
    sc = float(2.0 ** (-FP16_SHIFT))

    with tile.TileContext(nc) as tc:
        with (
            tc.tile_pool(name="acts", bufs=1) as acts,
            tc.tile_pool(name="w16pool", bufs=3) as w16pool,
            tc.tile_pool(name="w8pool", bufs=3) as w8pool,
            tc.tile_pool(name="tmp", bufs=3) as tpool,
            tc.tile_pool(name="const", bufs=1) as cpool,
            tc.tile_pool(name="psumh", bufs=3, space="PSUM") as pph,
            tc.tile_pool(name="psuml", bufs=3, space="PSUM") as ppl,
            tc.tile_pool(name="zpsum", bufs=1, space="PSUM") as zppool,
            tc.tile_pool(name="zsb", bufs=2) as zsbpool,
        ):
            A_hi = acts.tile([P, NT * NB], FP16, name="A_hi")
            A_lo = acts.tile([P, NT * NB], FP8, name="A_lo")
            B_hi = acts.tile([P, NT * NB], FP16, name="B_hi")
            B_lo = acts.tile([P, NT * NB], FP8, name="B_lo")
            bias_sb = cpool.tile([P, L * NT], F32, name="bias_sb")
            wo16_sb = cpool.tile([P, NT * 2], FP16, name="wo16_sb")
            wo8_sb = cpool.tile([P, NT * 2], FP8, name="wo8_sb")
            nc.sync.dma_start(bias_sb[:], bias_d[:])
            nc.sync.dma_start(wo16_sb[:], wo16_d[:])
            nc.sync.dma_start(wo8_sb[:], wo8_d[:])

            for c in range(CH):
                nc.sync.dma_start(A_hi[:], xth_d[c])
                nc.sync.dma_start(A_lo[:], xtl_d[c])
                for l in range(L):
                    ain_hi, ain_lo = (A_hi, A_lo) if l % 2 == 0 else (B_hi, B_lo)
                    aout_hi, aout_lo = (B_hi, B_lo) if l % 2 == 0 else (A_hi, A_lo)
                    for ko in range(NT):
                        slab16 = w16pool.tile([P, NT * P], FP16, name="w16slab")
                        nc.sync.dma_start(slab16[:], wt16_d[l, ko])
                        slab8 = w8pool.tile([P, NT * P], FP8, name="w8slab")
                        nc.sync.dma_start(slab8[:], wt8_d[l, ko])
                        # NOTE: batched ordering (all fp16, then all DR) measures
                        # faster than hi,hi,lo interleave (219.4 vs 224.2 ns/MM):
                        # alternating Normal/DoubleRow perf modes thrashes the
                        # PE weight path more than the DR LDWEIGHTS costs.
                        pt = pph.tile([P, NB], F32, name="pt")
                        for d in range(NT):
                            nc.tensor.matmul(
                                pt[:], slab16[:, d * P : (d + 1) * P],
                                ain_hi[:, d * NB : (d + 1) * NB],
                                start=(d == 0), stop=(d == NT - 1),
                            )
                        plo = ppl.tile([P, NB], F32, name="plo")
                        for m in range(NT // 2):
                            lhsT = slab8[:, 2 * m * P : (2 * m + 2) * P].rearrange(
                                "p (j c) -> p j c", j=2
                            )
                            rhs = ain_lo[
                                :, 2 * m * NB : (2 * m + 2) * NB
                            ].rearrange("p (j b) -> p j b", j=2)
                            nc.tensor.matmul(
                                plo[:], lhsT, rhs,
                                start=(m == 0), stop=(m == NT // 2 - 1),
                                perf_mode=mybir.MatmulPerfMode.DoubleRow,
                            )
                        # combine + relu + re-split (t4s is relu result x LOSC)
                        t1 = tpool.tile([P, NB], F32, name="t1")
                        nc.scalar.mul(t1[:], plo[:], 1.0 / LOSC)
                        t2 = tpool.tile([P, NB], F32, name="t2")
                        nc.vector.tensor_add(t2[:], t1[:], pt[:])
                        t4s = tpool.tile([P, NB], F32, name="t4s")
                        nc.scalar.activation(
                            t4s[:], t2[:], mybir.ActivationFunctionType.Relu,
                            bias=bias_sb[:, l * NT + ko : l * NT + ko + 1],
                            scale=sc * LOSC,
                        )
                        nc.vector.tensor_scalar_mul(
                            aout_hi[:, ko * NB : (ko + 1) * NB], t4s[:], 1.0 / LOSC
                        )
                        nc.vector.scalar_tensor_tensor(
                            aout_lo[:, ko * NB : (ko + 1) * NB],
                            aout_hi[:, ko * NB : (ko + 1) * NB], -LOSC, t4s[:],
                            op0=mybir.AluOpType.mult, op1=mybir.AluOpType.add,
                        )

                hout_hi, hout_lo = (A_hi, A_lo) if L % 2 == 0 else (B_hi, B_lo)
                zph = zppool.tile([2, NB], F32, name="zph")
                for t in range(NT):
                    nc.tensor.matmul(
                        zph[:], wo16_sb[:, t * 2 : (t + 1) * 2],
                        hout_hi[:, t * NB : (t + 1) * NB],
                        start=(t == 0), stop=(t == NT - 1),
                    )
                zpl = zppool.tile([2, NB], F32, name="zpl")
                for t in range(NT):
                    nc.tensor.matmul(
                        zpl[:], wo8_sb[:, t * 2 : (t + 1) * 2],
                        hout_lo[:, t * NB : (t + 1) * NB],
                        start=(t == 0), stop=(t == NT - 1),
                    )
                zs1 = zsbpool.tile([2, NB], F32, name="zs1")
                nc.scalar.mul(zs1[:], zpl[:], 1.0 / LOSC)
                zs = zsbpool.tile([2, NB], F32, name="zs")
                nc.vector.tensor_add(zs[:], zs1[:], zph[:])
                nc.sync.dma_start(z_d[:, c * NB : (c + 1) * NB], zs[:])

    nc.compile()
    return nc


def _build_strassen():
    """fp16 single-pass with one level of Strassen on each hidden layer.

    Per layer, per 512-batch chunk: W@h is computed as 7 half-size products
    M_m = Ac_m @ Bc_m with Ac ([2048,2048], values in {-2..2}, exact fp16)
    precomputed on host, Bc (activation combos, [2048k, 256n]) computed on
    the Vector engine, and C-quadrant recombination + relu on Scalar/Vector
    reading PSUM. PE work drops 12.5% (7/8) vs the direct 32x32-tile loop.

    Loop order is chunk-pipelined (c0's full layer, then c1) with next
    chunk/layer B-combos emitted early so DVE overlaps them under PE work.
    Activations live in-place in X[c]; MMs only read the Bc copies.
    """
    NT2 = NT // 2   # 16 k/out tiles per half
    NQ = NB // 2    # 256-batch quarter (Strassen N-split)
    nc = bacc.Bacc("TRN2", target_bir_lowering=False, debug=False)
    xt_d = nc.dram_tensor("xt", [CH, P, NT * NB], FP16, kind="ExternalInput").ap()
    # 7 A-combos per layer: [L, 7, ot(16), P, kt(16)*P]
    wt_d = nc.dram_tensor("wt", [L, 7, NT2, P, NT2 * P], FP16, kind="ExternalInput").ap()
    bias_d = nc.dram_tensor("biasd", [P, L * NT], F32, kind="ExternalInput").ap()
    wo_d = nc.dram_tensor("wo", [P, NT * 2], FP16, kind="ExternalInput").ap()
    z_d = nc.dram_tensor("z", [2, BC], F32, kind="ExternalOutput").ap()
    sc = float(2.0 ** (-FP16_SHIFT))

    with tile.TileContext(nc) as tc:
        with (
            tc.tile_pool(name="acts", bufs=1) as acts,
            tc.tile_pool(name="bcpool", bufs=1) as bcpool,
            tc.tile_pool(name="wpool", bufs=4) as wpool,
            tc.tile_pool(name="tmp", bufs=8) as tpool,
            tc.tile_pool(name="const", bufs=1) as cpool,
            tc.tile_pool(name="psum", bufs=10, space="PSUM") as ppool,
            tc.tile_pool(name="zpsum", bufs=2, space="PSUM") as zppool,
            tc.tile_pool(name="zsb", bufs=2) as zsbpool,
        ):
            X = [acts.tile([P, NT * NB], FP16, name=f"X{c}") for c in range(CH)]
            # 7 B-combo tiles per chunk, [128, kt(16) * 256]
            Bc = [[bcpool.tile([P, NT2 * NQ], FP16, name=f"Bc{c}_{m}")
                   for m in range(7)] for c in range(CH)]
            bias_sb = cpool.tile([P, L * NT], F32, name="bias_sb")
            wo_sb = cpool.tile([P, NT * 2], FP16, name="wo_sb")
            nc.sync.dma_start(bias_sb[:], bias_d[:])
            nc.sync.dma_start(wo_sb[:], wo_d[:])
            for c in range(CH):
                nc.sync.dma_start(X[c][:], xt_d[c])

            def emit_combos(c):
                # h viewed as [p, ktile(32), batch(512)]
                hv = X[c][:].rearrange("p (d n) -> p d n", d=NT)
                B11 = hv[:, 0:NT2, 0:NQ]
                B12 = hv[:, 0:NT2, NQ:NB]
                B21 = hv[:, NT2:NT, 0:NQ]
                B22 = hv[:, NT2:NT, NQ:NB]
                Bv = [Bc[c][m][:].rearrange("p (d n) -> p d n", d=NT2)
                      for m in range(7)]
                nc.vector.tensor_add(Bv[0], B11, B22)   # M1: B11+B22
                nc.vector.tensor_copy(Bv[1], B11)       # M2: B11
                nc.vector.tensor_sub(Bv[2], B12, B22)   # M3: B12-B22
                nc.vector.tensor_sub(Bv[3], B21, B11)   # M4: B21-B11
                nc.vector.tensor_copy(Bv[4], B22)       # M5: B22
                nc.vector.tensor_add(Bv[5], B11, B12)   # M6: B11+B12
                nc.vector.tensor_add(Bv[6], B21, B22)   # M7: B21+B22

            for l in range(L):
                for c in range(CH):
                    emit_combos(c)
                for c in range(CH):
                    for ot in range(NT2):
                        ps = []
                        for m in range(7):
                            slab = wpool.tile([P, NT2 * P], FP16, name="wslab")
                            nc.sync.dma_start(slab[:], wt_d[l, m, ot])
                            pt = ppool.tile([P, NQ], F32, name="pt")
                            for k in range(NT2):
                                nc.tensor.matmul(
                                    pt[:], slab[:, k * P : (k + 1) * P],
                                    Bc[c][m][:, k * NQ : (k + 1) * NQ],
                                    start=(k == 0), stop=(k == NT2 - 1),
                                )
                            ps.append(pt)
                        M1, M2, M3, M4, M5, M6, M7 = ps
                        # SBUF staging copies (ACT engine, PSUM->SBUF)
                        s1 = tpool.tile([P, NQ], F32, name="s1")
                        nc.scalar.copy(s1[:], M1[:])
                        s2 = tpool.tile([P, NQ], F32, name="s2")
                        nc.scalar.copy(s2[:], M2[:])
                        s3 = tpool.tile([P, NQ], F32, name="s3")
                        nc.scalar.copy(s3[:], M3[:])
                        s4 = tpool.tile([P, NQ], F32, name="s4")
                        nc.scalar.copy(s4[:], M4[:])
                        bias_u = bias_sb[:, l * NT + ot : l * NT + ot + 1]
                        bias_d_ = bias_sb[:, l * NT + NT2 + ot : l * NT + NT2 + ot + 1]
                        # C11 = M1+M4-M5+M7 -> X[:, ot, 0:NQ]
                        u = tpool.tile([P, NQ], F32, name="u")
                        nc.vector.tensor_add(u[:], s1[:], s4[:])
                        nc.vector.tensor_sub(u[:], u[:], M5[:])
                        nc.vector.tensor_add(u[:], u[:], M7[:])
                        nc.scalar.activation(
                            X[c][:, ot * NB : ot * NB + NQ], u[:],
                            mybir.ActivationFunctionType.Relu,
                            bias=bias_u, scale=sc,
                        )
                        # C12 = M3+M5 -> X[:, ot, NQ:NB]
                        v = tpool.tile([P, NQ], F32, name="v")
                        nc.vector.tensor_add(v[:], s3[:], M5[:])
                        nc.scalar.activation(
                            X[c][:, ot * NB + NQ : (ot + 1) * NB], v[:],
                            mybir.ActivationFunctionType.Relu,
                            bias=bias_u, scale=sc,
                        )
                        # C21 = M2+M4 -> X[:, 16+ot, 0:NQ]
                        w = tpool.tile([P, NQ], F32, name="w")
                        nc.vector.tensor_add(w[:], s2[:], s4[:])
                        nc.scalar.activation(
                            X[c][:, (NT2 + ot) * NB : (NT2 + ot) * NB + NQ], w[:],
                            mybir.ActivationFunctionType.Relu,
                            bias=bias_d_, scale=sc,
                        )
                        # C22 = M1-M2+M3+M6 -> X[:, 16+ot, NQ:NB]
                        y = tpool.tile([P, NQ], F32, name="y")
                        nc.vector.tensor_sub(y[:], s1[:], s2[:])
                        nc.vector.tensor_add(y[:], y[:], s3[:])
                        nc.vector.tensor_add(y[:], y[:], M6[:])
                        nc.scalar.activation(
                            X[c][:, (NT2 + ot) * NB + NQ : (NT2 + ot + 1) * NB], y[:],
                            mybir.ActivationFunctionType.Relu,
                            bias=bias_d_, scale=sc,
                        )

            for c in range(CH):
                zp = zppool.tile([2, NB], F32, name="zp")
                for t in range(NT):
                    nc.tensor.matmul(
                        zp[:], wo_sb[:, t * 2 : (t + 1) * 2],
                        X[c][:, t * NB : (t + 1) * NB],
                        start=(t == 0), stop=(t == NT - 1),
                    )
                zs = zsbpool.tile([2, NB], F32, name="zs")
                nc.vector.tensor_copy(zs[:], zp[:])
                nc.sync.dma_start(z_d[:, c * NB : (c + 1) * NB], zs[:])

    nc.compile()
    return nc


def _build_program(mode):
    if mode not in _BUILD_CACHE:
        _BUILD_CACHE[mode] = {
            "split2": _build_split2, "fp16": _build_fp16, "hybrid": _build_hybrid,
            "strassen": _build_strassen,
        }[mode]()
    return _BUILD_CACHE[mode]


def _split_hilo(a32: np.ndarray):
    hi = a32.astype(BF)
    lo = (a32 - hi.astype(np.float32)).astype(BF)
    return hi, lo


def _weights_layout(w: np.ndarray) -> np.ndarray:
    """[dout, din] f32 -> [NT(ko), P(p), NT*P(t*128+c)]."""
    return w.reshape(NT, P, NT, P).transpose(0, 3, 2, 1).reshape(NT, P, NT * P)


def _prepare_inputs(mode, x, scores1_h, scores2_h, bias_h, scores1_o, scores2_o):
    """Host-side: masks, weight/bias/x layouts for the device program.

    Returns a list of per-core in_maps (without only the core-varying xt)."""
    F8 = ml_dtypes.float8_e4m3
    wdt = BF if mode == "split2" else F16
    wt = np.empty((L, NT, P, NT * P), wdt)
    wt8 = np.empty((L, NT, P, NT * P), F8) if mode == "hybrid" else None
    for l in range(L):
        w = _subnet_mask(scores1_h[l]) - _subnet_mask(scores2_h[l])  # [dout, din]
        wl = _weights_layout(w)
        wt[l] = wl.astype(wdt)
        if wt8 is not None:
            wt8[l] = wl.astype(F8)

    b2 = 2.0 * bias_h.astype(np.float32)  # [L, D]
    if mode in ("fp16", "hybrid"):
        scales = (2.0 ** (-FP16_SHIFT * np.arange(1, L + 1, dtype=np.float32)))
        b2 = b2 * scales[:, None]
        if mode == "hybrid":
            b2 = b2 * LOSC
    bias_sb = np.ascontiguousarray(
        b2.reshape(L, NT, P).transpose(2, 0, 1).reshape(P, L * NT)
    )

    wo = _subnet_mask(scores1_o) - _subnet_mask(scores2_o)  # [2, D]
    wo_l = np.ascontiguousarray(
        wo.reshape(2, NT, P).transpose(2, 1, 0).reshape(P, NT * 2)
    )
    wo_sb = wo_l.astype(wdt)

    common = {}
    if mode == "hybrid":
        common = {"wt16": wt, "wt8": wt8, "biasd": bias_sb,
                  "wo16": wo_sb, "wo8": wo_l.astype(F8)}
    else:
        common = {"wt": wt, "biasd": bias_sb, "wo": wo_sb}

    in_maps = []
    for i in range(N_CORES):
        xT = x[i * BC : (i + 1) * BC].T.astype(np.float32)  # [D, BC]
        xr = xT.reshape(NT, P, CH, NB).transpose(2, 1, 0, 3).reshape(CH, P, NT * NB)
        xr = np.ascontiguousarray(xr)
        if mode == "split2":
            hi, lo = _split_hilo(xr)
            xt = {"xt": np.ascontiguousarray(np.stack([hi, lo], axis=1))}
        elif mode == "fp16":
            xt = {"xt": xr.astype(F16)}
        else:
            hi = xr.astype(F16)
            lo8 = ((xr - hi.astype(np.float32)) * LOSC).astype(F8)
            xt = {"xth": hi, "xtl": lo8}
        in_maps.append({**common, **xt})
    return in_maps


def kernel(x, scores1_h, scores2_h, bias_h, scores1_o, scores2_o, bias_o, w_last,
           _trace=False, _run_kwargs=None):
    x = np.asarray(x, np.float32)
    scores1_h = np.asarray(scores1_h, np.float32)
    scores2_h = np.asarray(scores2_h, np.float32)
    bias_h = np.asarray(bias_h, np.float32)
    scores1_o = np.asarray(scores1_o, np.float32)
    scores2_o = np.asarray(scores2_o, np.float32)
    bias_o = np.asarray(bias_o, np.float32)
    w_last = np.asarray(w_last, np.float32)

    in_maps = _prepare_inputs(
        MODE, x, scores1_h, scores2_h, bias_h, scores1_o, scores2_o
    )
    nc = _build_program(MODE)
    # Retry guard: very rare transient HW faults have been observed to produce
    # NaN output (z is tiny, so the check is free). Clean runs are bit-identical.
    for attempt in range(3):
        res = run_bass_kernel_spmd(
            nc, in_maps, core_ids=list(range(N_CORES)), trace=_trace,
            **(_run_kwargs or {}),
        )
        zs_all = np.stack([res.results[i]["z"] for i in range(N_CORES)])
        zbound = 1e11 if MODE == "split2" else 1e4  # z is 2^-30-scaled otherwise
        if np.isfinite(zs_all).all() and np.abs(zs_all).max() < zbound:
            break
        print(f"kernel: bad z detected (attempt {attempt}), retrying", file=sys.stderr)

    # host-side tail: relu(z.T + 2*b_o) @ w_last.T  (tiny: [8192, 2] -> [8192, 1])
    zscale = 1.0 if MODE == "split2" else float(2.0 ** (FP16_SHIFT * L))
    # (fp16/hybrid carry h5 scaled by 2^-30; z output is pre-bias, pre-relu)
    y = np.empty((B, 1), np.float32)
    for i in range(N_CORES):
        z = res.results[i]["z"].astype(np.float32)  # [2, BC]
        h = np.maximum(z.T * zscale + 2.0 * bias_o[None, :], 0.0).astype(np.float32)
        y[i * BC : (i + 1) * BC] = h @ w_last.T
    if _trace:
        kernel.last_results = res
    return y

